# revision 6
# baseline (speedup 1.0000x reference)
"""Trainium2 Bass kernel for nn_LpAlignEntropyLoss.

Loss over three views z1,z2,z3 (each [8192,128] f32):
  for each pair (i<j):
    pos += mean_m ||zi_m - zj_m||
    neg += mean_m [ ln(sum_n exp(-d_mn)) - ln(B) ],  d = cdist(zi, zj)
  loss = (0.5*pos + 0.5*neg) / 3

Strategy: shard the 8192 rows across 8 cores (1024 each). Per core:

  PE   : fp8(e4m3) DoubleRow matmuls compute the full squared distance
         q = a2[m] + b2[n] - 2*zi.zj in ONE pass: k-tile 0 carries
         (-sqrt2*zi) x (sqrt2*zj), k-tile 1 carries the norm terms as
         constant-vector outer products (b2 in 3 fp8 rows against ones,
         a2's 3 fp8 rows against a constant rhs), so no separate bias
         matmuls are needed.  0.5 cycles/row -> ~41us.
  ACT  : single Sqrt pass PSUM f32 -> fp16 d tiles (the only PSUM->SBUF
         crossing; ACT is the bottleneck at ~178us).
  DVE  : one 4x tensor_scalar per row computes the fp16 BIT PATTERN of
         exp((S-d)/tau) via the Schraudolph trick (bits16(2^y) is linear
         in y up to a +-3% mantissa wiggle), writing uint16; a second 4x
         tensor_scalar re-reads the same bytes bitcast to fp16 and
         row-accumulates.
  Host : the +-3% wiggle, the fp8 quantization distance shift, the fp16
         rounding and the ACT sqrt-table error are all removed by an
         on-device calibration: sampled q values go through the SAME
         sqrt->exp-bits->reduce pipeline, and the ratio to their exact
         host sums gives one correction scalar per pair.  The positive-
         pair term is O(B*D) and computed exactly on host.
"""

import math

import numpy as np
import ml_dtypes

import concourse.bacc as bacc
import concourse.bass as bass
import concourse.mybir as mybir
import concourse.tile as tile
from concourse.bass_utils import run_bass_kernel_spmd

B, D = 8192, 128
NCORES = 8
ML = B // NCORES          # rows per core (1024)
MB = ML // 128            # m-blocks per core (8)
PAIRS = [(0, 1), (0, 2), (1, 2)]
TAU = 1.0
ALPHA = 0.5
S_SHIFT = 12.0            # exp((S-d)/tau): keeps fp16 bits in [3k, 16k]
LOG2E = float(np.log2(np.e))
C0E = -1024.0 * LOG2E / TAU
C1E = 1024.0 * (S_SHIFT * LOG2E / TAU + 15.0)
CALN = 512                # calibration columns per pair per core

F32 = mybir.dt.float32
FP16 = mybir.dt.float16
U16 = mybir.dt.uint16
FP8 = mybir.dt.float8e4
AF = mybir.ActivationFunctionType
ALU = mybir.AluOpType
SQ2 = math.sqrt(2.0)

RHS_VIEWS = (1, 2)        # views used as cdist columns
LHS_VIEWS = (0, 1)        # views used as cdist rows
PAIR_LHS = {0: 0, 1: 0, 2: 1}
PAIR_RHS = {0: 1, 1: 2, 2: 2}


def build(nc: bacc.Bacc):
    rhs_in = {j: nc.dram_tensor(f"rhs{j}", [128, 2, B], FP8, kind="ExternalInput")
              for j in RHS_VIEWS}
    lhs_in = {i: nc.dram_tensor(f"lhs{i}", [128, 2, ML], FP8, kind="ExternalInput")
              for i in LHS_VIEWS}
    calq_in = nc.dram_tensor("calq", [128, 3, CALN], F32, kind="ExternalInput")
    out = nc.dram_tensor("out", [128, 32], F32, kind="ExternalOutput")

    with tile.TileContext(nc) as tc:
        with tc.tile_pool(name="persist", bufs=1) as persist:
            rt = {j: persist.tile([128, 2, B], FP8, tag=f"rt{j}", name=f"rt{j}") for j in RHS_VIEWS}
            lt = {i: persist.tile([128, 2, ML], FP8, tag=f"lt{i}", name=f"lt{i}") for i in LHS_VIEWS}
            cq = persist.tile([128, 3, CALN], F32, tag="cq", name="cq")
            sacc = persist.tile([128, 32], F32, tag="sacc", name="sacc")

            # first pair (0,1) needs lhs0 + leading rhs1 columns; load those
            # first so the PE/ACT pipeline starts ~5us earlier.
            nc.sync.dma_start(lt[0][:], lhs_in[0][:])
            for c0, c1 in ((0, 512), (512, 1536), (1536, 3584), (3584, 8192)):
                nc.sync.dma_start(rt[1][:, :, c0:c1], rhs_in[1][:, :, c0:c1])
            nc.sync.dma_start(cq[:], calq_in[:])
            nc.sync.dma_start(lt[1][:], lhs_in[1][:])
            for cdma in range(4):
                nc.sync.dma_start(rt[2][:, :, cdma * 2048:(cdma + 1) * 2048],
                                  rhs_in[2][:, :, cdma * 2048:(cdma + 1) * 2048])

            with (
                tc.tile_pool(name="mpsum", bufs=2, space="PSUM") as mpsum,
                tc.tile_pool(name="dpool", bufs=3) as dpool,
                tc.tile_pool(name="epool", bufs=2) as epool,
                tc.tile_pool(name="spool", bufs=1) as spool,
            ):
                scr = spool.tile([128, B], FP16, tag="scr", name="scr")

                def calib_jobs():
                    # same sqrt -> exp-bits -> reduce pipeline on sampled q
                    # values (ACT reads SBUF f32 here).
                    for p in range(3):
                        dcal = dpool.tile([128, CALN], FP16, tag="dcal", name="dcal")
                        nc.scalar.activation(dcal[:], cq[:, p, :], AF.Sqrt)
                        ebc = epool.tile([128, CALN], U16, tag="ec", name="ec")
                        nc.vector.tensor_scalar(ebc[:], dcal[:], C0E, C1E,
                                                ALU.mult, ALU.add)
                        nc.vector.tensor_scalar(scr[:, 0:CALN], ebc[:].bitcast(FP16),
                                                1.0, 0.0, ALU.mult, ALU.add,
                                                accum_out=sacc[:, 24 + p:24 + p + 1])

                col = 0
                njobs = len(PAIRS) * MB
                for p, (i, j) in enumerate(PAIRS):
                    for k in range(MB):
                        last = (col == njobs - 1)
                        dt = dpool.tile([128, B], FP16, tag="d", name="d")
                        lhsT = lt[i][:, :, k * 128:(k + 1) * 128]
                        for c4 in range(4):
                            ps = mpsum.tile([128, 2048], F32, tag="mm", name="mm")
                            for s in range(4):
                                n0 = c4 * 2048 + s * 512
                                nc.tensor.matmul(
                                    ps[:, s * 512:(s + 1) * 512], lhsT,
                                    rt[j][:, :, n0:n0 + 512],
                                    start=True, stop=True,
                                    perf_mode=mybir.MatmulPerfMode.DoubleRow)
                            nc.scalar.activation(
                                dt[:, c4 * 2048:(c4 + 1) * 2048], ps[:], AF.Sqrt)
                            if last:
                                # drain the final row chunk-by-chunk so the DVE
                                # tail overlaps the last ACT chunks
                                sl = slice(c4 * 2048, (c4 + 1) * 2048)
                                eb = epool.tile([128, 2048], U16, tag="el", name="el")
                                nc.vector.tensor_scalar(eb[:], dt[:, sl], C0E, C1E,
                                                        ALU.mult, ALU.add)
                                nc.vector.tensor_scalar(
                                    scr[:, sl], eb[:].bitcast(FP16),
                                    1.0, 0.0, ALU.mult, ALU.add,
                                    accum_out=sacc[:, 27 + c4:28 + c4])
                        if not last:
                            eb = epool.tile([128, B], U16, tag="e", name="e")
                            nc.vector.tensor_scalar(eb[:], dt[:], C0E, C1E,
                                                    ALU.mult, ALU.add)
                            nc.vector.tensor_scalar(scr[:], eb[:].bitcast(FP16),
                                                    1.0, 0.0, ALU.mult, ALU.add,
                                                    accum_out=sacc[:, col:col + 1])
                        col += 1
                        if col == 1:
                            calib_jobs()

            nc.sync.dma_start(out[:], sacc[:])
    return nc


def _q8(a):
    return np.asarray(a, dtype=np.float32).astype(ml_dtypes.float8_e4m3)


def _decomp3(v, first_half=False):
    """Decompose f64 vector v into 3 fp8 rows (r1[*2 if first_half] + r2 + r3)."""
    f64 = np.float64
    if first_half:
        r1 = _q8(v / 2)
        rem = v - 2.0 * r1.astype(f64)
    else:
        r1 = _q8(v)
        rem = v - r1.astype(f64)
    r2 = _q8(rem)
    rem = rem - r2.astype(f64)
    r3 = _q8(rem)
    resid = rem - r3.astype(f64)
    return r1, r2, r3, resid


_CACHE = {}


def kernel(z1: np.ndarray, z2: np.ndarray, z3: np.ndarray) -> np.ndarray:
    f64 = np.float64
    zs = [np.asarray(z, dtype=np.float32) for z in (z1, z2, z3)]
    zT = [np.ascontiguousarray(z.T) for z in zs]            # [128, 8192] f32
    zT64 = [t.astype(f64) for t in zT]

    # fp8 quantizations actually fed to the PE
    rhs0 = {j: _q8(SQ2 * zT[j]) for j in RHS_VIEWS}
    lhs0 = {i: _q8(-SQ2 * zT[i]) for i in LHS_VIEWS}
    eff_r = {j: rhs0[j].astype(f64) / SQ2 for j in RHS_VIEWS}
    eff_l = {i: lhs0[i].astype(f64) / -SQ2 for i in LHS_VIEWS}

    a2 = {i: (eff_l[i] ** 2).sum(0) for i in LHS_VIEWS}     # [8192] f64
    b2 = {j: (eff_r[j] ** 2).sum(0) for j in RHS_VIEWS}

    # aux fp8 rows; device-exact norm sums include the tiny fp8 residual
    g1, g2, g3 = {}, {}, {}
    a2_dev = {}
    for i in LHS_VIEWS:
        g1[i], g2[i], g3[i], res = _decomp3(a2[i], first_half=True)
        a2_dev[i] = a2[i] - res
    h1, h2, h3 = {}, {}, {}
    b2_dev = {}
    for j in RHS_VIEWS:
        h1[j], h2[j], h3[j], res = _decomp3(b2[j])
        b2_dev[j] = b2[j] - res

    rhs_tiles = {}
    for j in RHS_VIEWS:
        k1 = np.zeros((128, B), dtype=ml_dtypes.float8_e4m3)
        k1[0, :] = h1[j]; k1[1, :] = h2[j]; k1[2, :] = h3[j]
        k1[3, :] = 2.0;   k1[4, :] = 1.0;   k1[5, :] = 1.0
        rhs_tiles[j] = np.ascontiguousarray(np.stack([rhs0[j], k1], axis=1))

    lhs_tiles = {}
    for i in LHS_VIEWS:
        k1 = np.zeros((128, B), dtype=ml_dtypes.float8_e4m3)
        k1[0, :] = 1.0; k1[1, :] = 1.0; k1[2, :] = 1.0
        k1[3, :] = g1[i]; k1[4, :] = g2[i]; k1[5, :] = g3[i]
        lhs_tiles[i] = np.ascontiguousarray(np.stack([lhs0[i], k1], axis=1))

    # exact norms/dots of the ORIGINAL f32 inputs (f64 accumulation)
    nrm_true = [(t * t).sum(0) for t in zT64]

    # ---- positive-pair term: exact on host, O(B*D) ----
    pos_loss = sum(
        float(np.sqrt(np.maximum(
            nrm_true[i] + nrm_true[j] - 2.0 * (zT64[i] * zT64[j]).sum(0), 0.0)).mean())
        for i, j in PAIRS)

    # ---- calibration samples: device q-hat vs exact exp sums ----
    rng = np.random.default_rng(12345)
    NS = NCORES * CALN  # samples per pair (one [128, CALN/128... ] slice per core)
    calqs = []          # per core: [128, 3, CALN] f32
    true_sums = np.zeros((NCORES, 3))
    for p, (i, j) in enumerate(PAIRS):
        mi = rng.integers(0, B, size=128 * NS // 128 * 1)  # NS samples
        mi = rng.integers(0, B, size=NS)
        nj = rng.integers(0, B, size=NS)
        dot_eff = (eff_l[i][:, mi] * eff_r[j][:, nj]).sum(0)
        qhat = (a2_dev[i][mi] + b2_dev[j][nj] - 2.0 * dot_eff)
        dot_true = (zT64[i][:, mi] * zT64[j][:, nj]).sum(0)
        d_true = np.sqrt(np.maximum(
            nrm_true[i][mi] + nrm_true[j][nj] - 2.0 * dot_true, 0.0))
        ev = np.exp((S_SHIFT - d_true) / TAU)
        for c in range(NCORES):
            sl = slice(c * CALN, (c + 1) * CALN)
            if p == 0:
                calqs.append(np.zeros((128, 3, CALN), dtype=np.float32))
            calqs[c][:, p, :] = np.float32(qhat[sl])[None, :]
            true_sums[c, p] = 128.0 * ev[sl].sum()

    in_maps = []
    for c in range(NCORES):
        m = {f"rhs{j}": rhs_tiles[j] for j in RHS_VIEWS}
        for i in LHS_VIEWS:
            m[f"lhs{i}"] = np.ascontiguousarray(lhs_tiles[i][:, :, c * ML:(c + 1) * ML])
        m["calq"] = calqs[c]
        in_maps.append(m)

    if "nc" not in _CACHE:
        nc = bacc.Bacc("TRN2", target_bir_lowering=False)
        build(nc)
        nc.finalize()
        _CACHE["nc"] = nc
    nc = _CACHE["nc"]

    res = None
    for attempt in range(4):
        try:
            res = run_bass_kernel_spmd(nc, in_maps, core_ids=list(range(NCORES)))
            outs = [r["out"] for r in res.results]
            cal_dev = np.array([[o[:, 24 + p].sum() for p in range(3)] for o in outs])
            ratios = cal_dev / true_sums
            ok = (np.all(np.isfinite(ratios)) and np.all(ratios > 0.7)
                  and np.all(ratios < 1.5)
                  and all(np.all(np.isfinite(o[:, :31])) and np.all(o[:, :23] > 0)
                          and np.all(o[:, 27:31] > 0) for o in outs))
        except Exception:
            ok = False
        if ok:
            break
        import time
        import jax
        try:
            jax.clear_backends()
        except Exception:
            pass
        time.sleep(8)
    assert res is not None
    _CACHE["last_res"] = res

    outs = [r["out"].astype(f64) for r in res.results]
    cal_dev = np.array([[o[:, 24 + p].sum() for p in range(3)] for o in outs])
    R = cal_dev.sum(0) / true_sums.sum(0)          # per-pair wiggle ratio


    # device col 23 (last row-job) was drained in 4 chunks into cols 27..30
    outs = [np.concatenate([o[:, :23], (o[:, 27:31].sum(1))[:, None]], axis=1)
            for o in outs]
    neg_loss = 0.0
    for p in range(3):
        svals = np.concatenate([o[:, p * MB:(p + 1) * MB].reshape(-1) for o in outs])
        lse = np.log(svals) - math.log(R[p]) - S_SHIFT / TAU
        neg_loss += float(lse.mean()) - math.log(B)

    loss = (ALPHA * pos_loss + (1.0 - ALPHA) * neg_loss) / len(PAIRS)
    return np.float32(loss)


# revision 15
# speedup vs baseline: 1.0243x; 1.0243x over previous
"""Trainium2 Bass kernel for nn_LpAlignEntropyLoss.

Loss over three views z1,z2,z3 (each [8192,128] f32):
  for each pair (i<j):
    pos += mean_m ||zi_m - zj_m||
    neg += mean_m [ ln(sum_n exp(-d_mn)) - ln(B) ],  d = cdist(zi, zj)
  loss = (0.5*pos + 0.5*neg) / 3

Strategy: shard the 8192 rows across 8 cores (1024 each). Per core:

  PE   : fp8(e4m3) DoubleRow matmuls compute the full squared distance
         q = a2[m] + b2[n] - 2*zi.zj in ONE pass: k-tile 0 carries
         (-sqrt2*zi) x (sqrt2*zj), k-tile 1 carries the norm terms as
         constant-vector outer products (b2 in 3 fp8 rows against ones,
         a2's 3 fp8 rows against a constant rhs), so no separate bias
         matmuls are needed.  0.5 cycles/row -> ~42us.
  ACT  : single Sqrt pass PSUM f32 -> fp16 d tiles (the only PSUM->SBUF
         crossing; ACT is the bottleneck at ~183us busy, ~95% of wall).
         Calibration runs first to fill the head DMA stall; the last two
         row-jobs drain chunk-wise so the DVE tail overlaps the final
         ACT chunks.
  DVE  : one 4x tensor_scalar per row computes the fp16 BIT PATTERN of
         exp((S-d)/tau) via the Schraudolph trick (bits16(2^y) is linear
         in y up to a +-3% mantissa wiggle), writing uint16; a second 4x
         tensor_scalar re-reads the same bytes bitcast to fp16 and
         row-accumulates.
  Host : the +-3% wiggle, the fp8 quantization distance shift, the fp16
         rounding and the ACT sqrt-table error are all removed by an
         on-device calibration: sampled q values go through the SAME
         sqrt->exp-bits->reduce pipeline, and the ratio to their exact
         host sums gives one correction scalar per pair.  The positive-
         pair term is O(B*D) and computed exactly on host.
"""

import math

import numpy as np
import ml_dtypes

import concourse.bacc as bacc
import concourse.mybir as mybir
import concourse.tile as tile
from concourse.bass_utils import run_bass_kernel_spmd

B, D = 8192, 128
NCORES = 8
ML = B // NCORES          # rows per core (1024)
MB = ML // 128            # m-blocks per core (8)
PAIRS = [(0, 1), (0, 2), (1, 2)]
TAU = 1.0
ALPHA = 0.5
S_SHIFT = 12.0            # exp((S-d)/tau): keeps fp16 bits in [3k, 16k]
LOG2E = float(np.log2(np.e))
C0E = -1024.0 * LOG2E / TAU
C1E = 1024.0 * (S_SHIFT * LOG2E / TAU + 15.0)
CALN = 128                # calibration columns per pair per core

F32 = mybir.dt.float32
FP16 = mybir.dt.float16
U16 = mybir.dt.uint16
FP8 = mybir.dt.float8e4
AF = mybir.ActivationFunctionType
ALU = mybir.AluOpType
SQ2 = math.sqrt(2.0)

RHS_VIEWS = (1, 2)        # views used as cdist columns
LHS_VIEWS = (0, 1)        # views used as cdist rows


def build(nc: bacc.Bacc):
    rhs_in = {j: nc.dram_tensor(f"rhs{j}", [128, 2, B], FP8, kind="ExternalInput")
              for j in RHS_VIEWS}
    lhs_in = {i: nc.dram_tensor(f"lhs{i}", [128, 2, ML], FP8, kind="ExternalInput")
              for i in LHS_VIEWS}
    calq_in = nc.dram_tensor("calq", [128, 3, CALN], F32, kind="ExternalInput")
    out = nc.dram_tensor("out", [128, 36], F32, kind="ExternalOutput")

    with tile.TileContext(nc) as tc:
        with tc.tile_pool(name="persist", bufs=1) as persist:
            rt = {j: persist.tile([128, 2, B], FP8, tag=f"rt{j}", name=f"rt{j}") for j in RHS_VIEWS}
            lt = {i: persist.tile([128, 2, ML], FP8, tag=f"lt{i}", name=f"lt{i}") for i in LHS_VIEWS}
            cq = persist.tile([128, 3, CALN], F32, tag="cq", name="cq")
            sacc = persist.tile([128, 36], F32, tag="sacc", name="sacc")

            # first pair (0,1) needs lhs0 + leading rhs1 columns; load those
            # first so the PE/ACT pipeline starts ~5us earlier.
            nc.sync.dma_start(lt[0][:], lhs_in[0][:])
            for c0, c1 in ((0, 2048), (2048, 8192)):
                nc.sync.dma_start(rt[1][:, :, c0:c1], rhs_in[1][:, :, c0:c1])
            nc.sync.dma_start(lt[1][:], lhs_in[1][:])
            for cdma in range(4):
                nc.sync.dma_start(rt[2][:, :, cdma * 2048:(cdma + 1) * 2048],
                                  rhs_in[2][:, :, cdma * 2048:(cdma + 1) * 2048])

            with (
                tc.tile_pool(name="mpsum", bufs=2, space="PSUM") as mpsum,
                tc.tile_pool(name="dpool", bufs=3) as dpool,
                tc.tile_pool(name="epool", bufs=2) as epool,
                tc.tile_pool(name="spool", bufs=1) as spool,
            ):
                scr = spool.tile([128, B], FP16, tag="scr", name="scr")

                def calib_jobs():
                    # same sqrt -> exp-bits -> reduce pipeline on sampled q
                    # values (ACT reads SBUF f32 here).
                    for p in range(3):
                        dcal = dpool.tile([128, CALN], FP16, tag="dcal", name="dcal")
                        nc.scalar.activation(dcal[:], cq[:, p, :], AF.Sqrt)
                        ebc = epool.tile([128, CALN], U16, tag="ec", name="ec")
                        nc.vector.tensor_scalar(ebc[:], dcal[:], C0E, C1E,
                                                ALU.mult, ALU.add)
                        nc.vector.tensor_scalar(scr[:, 0:CALN], ebc[:].bitcast(FP16),
                                                1.0, 0.0, ALU.mult, ALU.add,
                                                accum_out=sacc[:, 24 + p:24 + p + 1])

                calib_jobs()
                col = 0
                njobs = len(PAIRS) * MB
                for p, (i, j) in enumerate(PAIRS):
                    for k in range(MB):
                        last = (col >= njobs - 2)
                        dt = dpool.tile([128, B], FP16, tag="d", name="d")
                        lhsT = lt[i][:, :, k * 128:(k + 1) * 128]
                        for c4 in range(4):
                            ps = mpsum.tile([128, 2048], F32, tag="mm", name="mm")
                            for s in range(4):
                                n0 = c4 * 2048 + s * 512
                                nc.tensor.matmul(
                                    ps[:, s * 512:(s + 1) * 512], lhsT,
                                    rt[j][:, :, n0:n0 + 512],
                                    start=True, stop=True,
                                    perf_mode=mybir.MatmulPerfMode.DoubleRow)
                            if col == njobs - 1 and c4 == 3:
                                # split the very last chunk 1536+512 so the
                                # post-ACT drain chain is minimal
                                nc.scalar.activation(dt[:, 6144:7680], ps[:, 0:1536],
                                                     AF.Sqrt)
                                ebx = epool.tile([128, 1536], U16, tag="ex", name="ebx")
                                nc.vector.tensor_scalar(ebx[:], dt[:, 6144:7680],
                                                        C0E, C1E, ALU.mult, ALU.add)
                                nc.vector.tensor_scalar(
                                    scr[:, 6144:7680], ebx[:].bitcast(FP16),
                                    1.0, 0.0, ALU.mult, ALU.add,
                                    accum_out=sacc[:, 30:31])
                                nc.scalar.activation(dt[:, 7680:8192], ps[:, 1536:2048],
                                                     AF.Sqrt)
                                ebv = epool.tile([128, 512], U16, tag="ev", name="ebv")
                                nc.vector.tensor_scalar(ebv[:], dt[:, 7680:8192],
                                                        C0E, C1E, ALU.mult, ALU.add)
                                nc.vector.tensor_scalar(
                                    scr[:, 7680:8192], ebv[:].bitcast(FP16),
                                    1.0, 0.0, ALU.mult, ALU.add,
                                    accum_out=sacc[:, 35:36])
                                continue
                            nc.scalar.activation(
                                dt[:, c4 * 2048:(c4 + 1) * 2048], ps[:], AF.Sqrt)
                            if last:
                                # drain the final row chunk-by-chunk so the DVE
                                # tail overlaps the last ACT chunks
                                sl = slice(c4 * 2048, (c4 + 1) * 2048)
                                eb = epool.tile([128, 2048], U16, tag="el", name="el")
                                nc.vector.tensor_scalar(eb[:], dt[:, sl], C0E, C1E,
                                                        ALU.mult, ALU.add)
                                nc.vector.tensor_scalar(
                                    scr[:, sl], eb[:].bitcast(FP16),
                                    1.0, 0.0, ALU.mult, ALU.add,
                                    accum_out=sacc[:, 27 + 4 * (njobs - 1 - col) + c4:
                                                       28 + 4 * (njobs - 1 - col) + c4])
                        if not last:
                            eb = epool.tile([128, B], U16, tag="e", name="e")
                            nc.vector.tensor_scalar(eb[:], dt[:], C0E, C1E,
                                                    ALU.mult, ALU.add)
                            nc.vector.tensor_scalar(scr[:], eb[:].bitcast(FP16),
                                                    1.0, 0.0, ALU.mult, ALU.add,
                                                    accum_out=sacc[:, col:col + 1])
                        col += 1

            nc.sync.dma_start(out[:, 0:27], sacc[:, 0:27])
            nc.sync.dma_start(out[:, 27:32], sacc[:, 27:32])
    return nc


def _q8(a):
    return np.asarray(a, dtype=np.float32).astype(ml_dtypes.float8_e4m3)


def _decomp3(v, first_half=False):
    """Decompose f64 vector v into 3 fp8 rows (r1[*2 if first_half] + r2 + r3)."""
    f64 = np.float64
    if first_half:
        r1 = _q8(v / 2)
        rem = v - 2.0 * r1.astype(f64)
    else:
        r1 = _q8(v)
        rem = v - r1.astype(f64)
    r2 = _q8(rem)
    rem = rem - r2.astype(f64)
    r3 = _q8(rem)
    resid = rem - r3.astype(f64)
    return r1, r2, r3, resid


_CACHE = {}


def kernel(z1: np.ndarray, z2: np.ndarray, z3: np.ndarray) -> np.ndarray:
    f64 = np.float64
    zs = [np.asarray(z, dtype=np.float32) for z in (z1, z2, z3)]
    zT = [np.ascontiguousarray(z.T) for z in zs]            # [128, 8192] f32
    zT64 = [t.astype(f64) for t in zT]

    # fp8 quantizations actually fed to the PE
    rhs0 = {j: _q8(SQ2 * zT[j]) for j in RHS_VIEWS}
    lhs0 = {i: _q8(-SQ2 * zT[i]) for i in LHS_VIEWS}
    eff_r = {j: rhs0[j].astype(f64) / SQ2 for j in RHS_VIEWS}
    eff_l = {i: lhs0[i].astype(f64) / -SQ2 for i in LHS_VIEWS}

    a2 = {i: (eff_l[i] ** 2).sum(0) for i in LHS_VIEWS}     # [8192] f64
    b2 = {j: (eff_r[j] ** 2).sum(0) for j in RHS_VIEWS}

    # aux fp8 rows; device-exact norm sums include the tiny fp8 residual
    g1, g2, g3 = {}, {}, {}
    a2_dev = {}
    for i in LHS_VIEWS:
        g1[i], g2[i], g3[i], res = _decomp3(a2[i], first_half=True)
        a2_dev[i] = a2[i] - res
    h1, h2, h3 = {}, {}, {}
    b2_dev = {}
    for j in RHS_VIEWS:
        h1[j], h2[j], h3[j], res = _decomp3(b2[j])
        b2_dev[j] = b2[j] - res

    rhs_tiles = {}
    for j in RHS_VIEWS:
        k1 = np.zeros((128, B), dtype=ml_dtypes.float8_e4m3)
        k1[0, :] = h1[j]; k1[1, :] = h2[j]; k1[2, :] = h3[j]
        k1[3, :] = 2.0;   k1[4, :] = 1.0;   k1[5, :] = 1.0
        rhs_tiles[j] = np.ascontiguousarray(np.stack([rhs0[j], k1], axis=1))

    lhs_tiles = {}
    for i in LHS_VIEWS:
        k1 = np.zeros((128, B), dtype=ml_dtypes.float8_e4m3)
        k1[0, :] = 1.0; k1[1, :] = 1.0; k1[2, :] = 1.0
        k1[3, :] = g1[i]; k1[4, :] = g2[i]; k1[5, :] = g3[i]
        lhs_tiles[i] = np.ascontiguousarray(np.stack([lhs0[i], k1], axis=1))

    # exact norms/dots of the ORIGINAL f32 inputs (f64 accumulation)
    nrm_true = [(t * t).sum(0) for t in zT64]

    # ---- positive-pair term: exact on host, O(B*D) ----
    pos_loss = sum(
        float(np.sqrt(np.maximum(
            nrm_true[i] + nrm_true[j] - 2.0 * (zT64[i] * zT64[j]).sum(0), 0.0)).mean())
        for i, j in PAIRS)

    # ---- calibration samples: device q-hat vs exact exp sums ----
    rng = np.random.default_rng(12345)
    NS = NCORES * 128 * CALN   # distinct samples per pair (128 rows per core)
    calqs = []                 # per core: [128, 3, CALN] f32
    true_sums = np.zeros((NCORES, 3))
    for p, (i, j) in enumerate(PAIRS):
        mi = rng.integers(0, B, size=NS)
        nj = rng.integers(0, B, size=NS)
        dot_eff = (eff_l[i][:, mi] * eff_r[j][:, nj]).sum(0)
        qhat = (a2_dev[i][mi] + b2_dev[j][nj] - 2.0 * dot_eff)
        dot_true = (zT64[i][:, mi] * zT64[j][:, nj]).sum(0)
        d_true = np.sqrt(np.maximum(
            nrm_true[i][mi] + nrm_true[j][nj] - 2.0 * dot_true, 0.0))
        ev = np.exp((S_SHIFT - d_true) / TAU)
        per = 128 * CALN
        for c in range(NCORES):
            sl = slice(c * per, (c + 1) * per)
            if p == 0:
                calqs.append(np.zeros((128, 3, CALN), dtype=np.float32))
            calqs[c][:, p, :] = np.float32(qhat[sl]).reshape(128, CALN)
            true_sums[c, p] = ev[sl].sum()

    in_maps = []
    for c in range(NCORES):
        m = {f"rhs{j}": rhs_tiles[j] for j in RHS_VIEWS}
        for i in LHS_VIEWS:
            m[f"lhs{i}"] = np.ascontiguousarray(lhs_tiles[i][:, :, c * ML:(c + 1) * ML])
        m["calq"] = calqs[c]
        in_maps.append(m)

    if "nc" not in _CACHE:
        nc = bacc.Bacc("TRN2", target_bir_lowering=False)
        build(nc)
        nc.finalize()
        _CACHE["nc"] = nc
    nc = _CACHE["nc"]

    res = None
    for attempt in range(4):
        try:
            res = run_bass_kernel_spmd(nc, in_maps, core_ids=list(range(NCORES)))
            outs = [r["out"] for r in res.results]
            cal_dev = np.array([[o[:, 24 + p].sum() for p in range(3)] for o in outs])
            ratios = cal_dev / true_sums
            ok = (np.all(np.isfinite(ratios)) and np.all(ratios > 0.7)
                  and np.all(ratios < 1.5)
                  and all(np.all(np.isfinite(o)) and np.all(o[:, :22] > 0)
                          and np.all(o[:, 27:35] > 0) for o in outs))
        except Exception:
            ok = False
        if ok:
            break
        import time
        import jax
        try:
            jax.clear_backends()
        except Exception:
            pass
        time.sleep(8)
    assert res is not None
    _CACHE["last_res"] = res

    outs = [r["out"].astype(f64) for r in res.results]
    cal_dev = np.array([[o[:, 24 + p].sum() for p in range(3)] for o in outs])
    R = cal_dev.sum(0) / true_sums.sum(0)          # per-pair wiggle ratio

    # last two row-jobs drained chunk-wise: job 22 -> cols 31..34,
    # job 23 -> cols 27..30 plus col 35 (its final 512 slice)
    outs = [np.concatenate([o[:, :22], (o[:, 31:35].sum(1))[:, None],
                            (o[:, 27:31].sum(1) + o[:, 35])[:, None]], axis=1)
            for o in outs]
    neg_loss = 0.0
    for p in range(3):
        svals = np.concatenate([o[:, p * MB:(p + 1) * MB].reshape(-1) for o in outs])
        lse = np.log(svals) - math.log(R[p]) - S_SHIFT / TAU
        neg_loss += float(lse.mean()) - math.log(B)

    loss = (ALPHA * pos_loss + (1.0 - ALPHA) * neg_loss) / len(PAIRS)
    return np.float32(loss)


# revision 21
# speedup vs baseline: 1.0469x; 1.0221x over previous
"""Trainium2 Bass kernel for nn_LpAlignEntropyLoss.

Loss over three views z1,z2,z3 (each [8192,128] f32):
  for each pair (i<j):
    pos += mean_m ||zi_m - zj_m||
    neg += mean_m [ ln(sum_n exp(-d_mn)) - ln(B) ],  d = cdist(zi, zj)
  loss = (0.5*pos + 0.5*neg) / 3

Strategy: shard the 8192 rows across 8 cores (1024 each). Per core:

  PE   : fp8(e4m3) DoubleRow matmuls compute the full squared distance
         q = a2[m] + b2[n] - 2*zi.zj in ONE pass: k-tile 0 carries
         (-sqrt2*zi) x (sqrt2*zj), k-tile 1 carries the norm terms as
         constant-vector outer products (b2 in 3 fp8 rows against ones,
         a2's 3 fp8 rows against a constant rhs), so no separate bias
         matmuls are needed.  0.5 cycles/row -> ~42us.
  ACT  : single Sqrt pass PSUM f32 -> fp16 d tiles (the only PSUM->SBUF
         crossing; ACT is the bottleneck at ~183us busy, ~95% of wall).
         Calibration runs first to fill the head DMA stall; the last two
         row-jobs drain chunk-wise so the DVE tail overlaps the final
         ACT chunks.
  DVE  : one 4x tensor_scalar per row computes the fp16 BIT PATTERN of
         exp((S-d)/tau) via the Schraudolph trick (bits16(2^y) is linear
         in y up to a +-3% mantissa wiggle), writing uint16; a second 4x
         tensor_scalar re-reads the same bytes bitcast to fp16 and
         row-accumulates.
  Host : the +-3% wiggle, the fp8 quantization distance shift, the fp16
         rounding and the ACT sqrt-table error are all removed by an
         on-device calibration: sampled q values go through the SAME
         sqrt->exp-bits->reduce pipeline, and the ratio to their exact
         host sums gives one correction scalar per pair.  The positive-
         pair term is O(B*D) and computed exactly on host.
"""

import math

import numpy as np
import ml_dtypes

import concourse.bacc as bacc
import concourse.mybir as mybir
import concourse.tile as tile
from concourse.bass_utils import run_bass_kernel_spmd

B, D = 8192, 128
NCORES = 8
ML = B // NCORES          # rows per core (1024)
MB = ML // 128            # m-blocks per core (8)
PAIRS = [(0, 1), (0, 2), (1, 2)]
TAU = 1.0
ALPHA = 0.5
S_SHIFT = 12.0            # exp((S-d)/tau): keeps fp16 bits in [3k, 16k]
LOG2E = float(np.log2(np.e))
C0E = -1024.0 * LOG2E / TAU
C1E = 1024.0 * (S_SHIFT * LOG2E / TAU + 15.0)
CALN = 128                # calibration columns per pair per core

F32 = mybir.dt.float32
FP16 = mybir.dt.float16
U16 = mybir.dt.uint16
FP8 = mybir.dt.float8e4
AF = mybir.ActivationFunctionType
ALU = mybir.AluOpType
SQ2 = math.sqrt(2.0)

RHS_VIEWS = (1, 2)        # views used as cdist columns
LHS_VIEWS = (0, 1)        # views used as cdist rows


def build(nc: bacc.Bacc):
    rhs_in = {j: nc.dram_tensor(f"rhs{j}", [128, 2, B], FP8, kind="ExternalInput")
              for j in RHS_VIEWS}
    lhs_in = {i: nc.dram_tensor(f"lhs{i}", [128, 2, ML], FP8, kind="ExternalInput")
              for i in LHS_VIEWS}
    calq_in = nc.dram_tensor("calq", [128, 3, CALN], F32, kind="ExternalInput")
    out = nc.dram_tensor("out", [128, 40], F32, kind="ExternalOutput")

    with tile.TileContext(nc) as tc:
        with tc.tile_pool(name="persist", bufs=1) as persist:
            rt = {j: persist.tile([128, 2, B], FP8, tag=f"rt{j}", name=f"rt{j}") for j in RHS_VIEWS}
            lt = {i: persist.tile([128, 2, ML], FP8, tag=f"lt{i}", name=f"lt{i}") for i in LHS_VIEWS}
            cq = persist.tile([128, 3, CALN], F32, tag="cq", name="cq")
            sacc = persist.tile([128, 40], F32, tag="sacc", name="sacc")

            # first pair (0,1) needs lhs0 + leading rhs1 columns; load those
            # first so the PE/ACT pipeline starts ~5us earlier.
            nc.sync.dma_start(lt[0][:], lhs_in[0][:])
            for c0, c1 in ((0, 2048), (2048, 8192)):
                nc.sync.dma_start(rt[1][:, :, c0:c1], rhs_in[1][:, :, c0:c1])
            nc.sync.dma_start(lt[1][:], lhs_in[1][:])
            for cdma in range(4):
                nc.sync.dma_start(rt[2][:, :, cdma * 2048:(cdma + 1) * 2048],
                                  rhs_in[2][:, :, cdma * 2048:(cdma + 1) * 2048])

            with (
                tc.tile_pool(name="mpsum", bufs=2, space="PSUM") as mpsum,
                tc.tile_pool(name="dpool", bufs=4) as dpool,
                tc.tile_pool(name="epool", bufs=2) as epool,
                tc.tile_pool(name="spool", bufs=1) as spool,
                tc.tile_pool(name="qpool", bufs=2) as qpool,
                tc.tile_pool(name="napool", bufs=3) as napool,
                tc.tile_pool(name="nspool", bufs=2) as nspool,
            ):
                scr = spool.tile([128, B], FP16, tag="scr", name="scr")

                def calib_jobs():
                    # same sqrt -> exp-bits -> reduce pipeline on sampled q
                    # values (ACT reads SBUF f32 here).
                    for p in range(3):
                        dcal = dpool.tile([128, CALN], FP16, tag="dcal", name="dcal")
                        nc.scalar.activation(dcal[:], cq[:, p, :], AF.Sqrt)
                        ebc = epool.tile([128, CALN], U16, tag="ec", name="ec")
                        nc.vector.tensor_scalar(ebc[:], dcal[:], C0E, C1E,
                                                ALU.mult, ALU.add)
                        nc.vector.tensor_scalar(scr[:, 0:CALN], ebc[:].bitcast(FP16),
                                                1.0, 0.0, ALU.mult, ALU.add,
                                                accum_out=sacc[:, 24 + p:24 + p + 1])

                # Row-job OFF bypasses ACT entirely: its sqrt runs on the
                # DVE as a quadratic rsqrt seed (in u = q/512, fp16) plus one
                # 3rd-order Householder step, built from TS(4x)/TT(2x) ops.
                # The -0.0035 mean d-bias of the chain is folded into the
                # exp-bits add constant; the rest is absorbed by calibration.
                OFF = 11            # (p=1, k=3): lhs0 block 3 against rt2
                OFF_I, OFF_J, OFF_K = 0, 2, 3
                NB2, NB1, NB0 = 2.2808493, -4.3503131, 3.0564483
                C0E_OFF = C0E * float(np.sqrt(512.0))
                C1E_OFF = C1E + C0E * 0.00347

                def off_chunk(c4):
                    lhsT = lt[OFF_I][:, :, OFF_K * 128:(OFF_K + 1) * 128]
                    ps = mpsum.tile([128, 2048], F32, tag="mm", name="mm")
                    for s5 in range(4):
                        n0 = c4 * 2048 + s5 * 512
                        nc.tensor.matmul(ps[:, s5 * 512:(s5 + 1) * 512], lhsT,
                                         rt[OFF_J][:, :, n0:n0 + 512],
                                         start=True, stop=True,
                                         perf_mode=mybir.MatmulPerfMode.DoubleRow)
                    qo = qpool.tile([128, 2048], FP16, tag="qo", name="qo")
                    nc.vector.tensor_scalar(qo[:], ps[:], 1.0 / 512.0, 0.0,
                                            ALU.mult, ALU.add)
                    def na():
                        return napool.tile([128, 2048], FP16, tag="na", name="na")
                    u1 = na(); nc.vector.tensor_scalar(u1[:], qo[:], NB2, NB1, ALU.mult, ALU.add)
                    u2 = na(); nc.vector.tensor_tensor(u2[:], u1[:], qo[:], ALU.mult)
                    sd = nspool.tile([128, 2048], FP16, tag="ns", name="sd")
                    nc.vector.tensor_scalar(sd[:], u2[:], 1.0, NB0, ALU.mult, ALU.add)
                    t3 = na(); nc.vector.tensor_tensor(t3[:], sd[:], sd[:], ALU.mult)
                    w2 = na(); nc.vector.tensor_tensor(w2[:], t3[:], qo[:], ALU.mult)
                    hv = na(); nc.vector.tensor_scalar(hv[:], w2[:], 1.0, -5.0 / 3.0, ALU.mult, ALU.add)
                    hs = na(); nc.vector.tensor_tensor(hs[:], hv[:], hv[:], ALU.mult)
                    h3 = na(); nc.vector.tensor_scalar(h3[:], hs[:], 0.375, 1.875 - 0.375 * 25.0 / 9.0, ALU.mult, ALU.add)
                    s1 = na(); nc.vector.tensor_tensor(s1[:], sd[:], h3[:], ALU.mult)
                    d16 = qpool.tile([128, 2048], FP16, tag="od", name="d16")
                    nc.vector.tensor_tensor(d16[:], qo[:], s1[:], ALU.mult)
                    eb = epool.tile([128, 2048], U16, tag="el", name="eb")
                    nc.vector.tensor_scalar(eb[:], d16[:], C0E_OFF, C1E_OFF,
                                            ALU.mult, ALU.add)
                    nc.vector.tensor_scalar(scr[:, c4 * 2048:(c4 + 1) * 2048],
                                            eb[:].bitcast(FP16), 1.0, 0.0,
                                            ALU.mult, ALU.add,
                                            accum_out=sacc[:, 36 + c4:37 + c4])

                calib_jobs()
                col = 0
                njobs = len(PAIRS) * MB
                for p, (i, j) in enumerate(PAIRS):
                    for k in range(MB):
                        if col == OFF:
                            col += 1
                            continue
                        last = (col >= njobs - 2)
                        dt = dpool.tile([128, B], FP16, tag="d", name="d")
                        lhsT = lt[i][:, :, k * 128:(k + 1) * 128]
                        for c4 in range(4):
                            ps = mpsum.tile([128, 2048], F32, tag="mm", name="mm")
                            for s in range(4):
                                n0 = c4 * 2048 + s * 512
                                nc.tensor.matmul(
                                    ps[:, s * 512:(s + 1) * 512], lhsT,
                                    rt[j][:, :, n0:n0 + 512],
                                    start=True, stop=True,
                                    perf_mode=mybir.MatmulPerfMode.DoubleRow)
                            if col == njobs - 1 and c4 == 3:
                                # split the very last chunk 1536+512 so the
                                # post-ACT drain chain is minimal
                                nc.scalar.activation(dt[:, 6144:7680], ps[:, 0:1536],
                                                     AF.Sqrt)
                                ebx = epool.tile([128, 1536], U16, tag="ex", name="ebx")
                                nc.vector.tensor_scalar(ebx[:], dt[:, 6144:7680],
                                                        C0E, C1E, ALU.mult, ALU.add)
                                nc.vector.tensor_scalar(
                                    scr[:, 6144:7680], ebx[:].bitcast(FP16),
                                    1.0, 0.0, ALU.mult, ALU.add,
                                    accum_out=sacc[:, 30:31])
                                nc.scalar.activation(dt[:, 7680:8192], ps[:, 1536:2048],
                                                     AF.Sqrt)
                                ebv = epool.tile([128, 512], U16, tag="ev", name="ebv")
                                nc.vector.tensor_scalar(ebv[:], dt[:, 7680:8192],
                                                        C0E, C1E, ALU.mult, ALU.add)
                                nc.vector.tensor_scalar(
                                    scr[:, 7680:8192], ebv[:].bitcast(FP16),
                                    1.0, 0.0, ALU.mult, ALU.add,
                                    accum_out=sacc[:, 35:36])
                                continue
                            nc.scalar.activation(
                                dt[:, c4 * 2048:(c4 + 1) * 2048], ps[:], AF.Sqrt)
                            if last:
                                # drain the final row chunk-by-chunk so the DVE
                                # tail overlaps the last ACT chunks
                                sl = slice(c4 * 2048, (c4 + 1) * 2048)
                                eb = epool.tile([128, 2048], U16, tag="el", name="el")
                                nc.vector.tensor_scalar(eb[:], dt[:, sl], C0E, C1E,
                                                        ALU.mult, ALU.add)
                                nc.vector.tensor_scalar(
                                    scr[:, sl], eb[:].bitcast(FP16),
                                    1.0, 0.0, ALU.mult, ALU.add,
                                    accum_out=sacc[:, 27 + 4 * (njobs - 1 - col) + c4:
                                                       28 + 4 * (njobs - 1 - col) + c4])
                        if col in (3, 7, 13, 17):
                            off_chunk({3: 0, 7: 1, 13: 2, 17: 3}[col])
                        if not last:
                            eb = epool.tile([128, B], U16, tag="e", name="e")
                            nc.vector.tensor_scalar(eb[:], dt[:], C0E, C1E,
                                                    ALU.mult, ALU.add)
                            nc.vector.tensor_scalar(scr[:], eb[:].bitcast(FP16),
                                                    1.0, 0.0, ALU.mult, ALU.add,
                                                    accum_out=sacc[:, col:col + 1])
                        col += 1

            nc.sync.dma_start(out[:, 0:27], sacc[:, 0:27])
            nc.sync.dma_start(out[:, 27:32], sacc[:, 27:32])
    return nc


def _q8(a):
    return np.asarray(a, dtype=np.float32).astype(ml_dtypes.float8_e4m3)


def _decomp3(v, first_half=False):
    """Decompose f64 vector v into 3 fp8 rows (r1[*2 if first_half] + r2 + r3)."""
    f64 = np.float64
    if first_half:
        r1 = _q8(v / 2)
        rem = v - 2.0 * r1.astype(f64)
    else:
        r1 = _q8(v)
        rem = v - r1.astype(f64)
    r2 = _q8(rem)
    rem = rem - r2.astype(f64)
    r3 = _q8(rem)
    resid = rem - r3.astype(f64)
    return r1, r2, r3, resid


_CACHE = {}


def kernel(z1: np.ndarray, z2: np.ndarray, z3: np.ndarray) -> np.ndarray:
    f64 = np.float64
    zs = [np.asarray(z, dtype=np.float32) for z in (z1, z2, z3)]
    zT = [np.ascontiguousarray(z.T) for z in zs]            # [128, 8192] f32
    zT64 = [t.astype(f64) for t in zT]

    # fp8 quantizations actually fed to the PE
    rhs0 = {j: _q8(SQ2 * zT[j]) for j in RHS_VIEWS}
    lhs0 = {i: _q8(-SQ2 * zT[i]) for i in LHS_VIEWS}
    eff_r = {j: rhs0[j].astype(f64) / SQ2 for j in RHS_VIEWS}
    eff_l = {i: lhs0[i].astype(f64) / -SQ2 for i in LHS_VIEWS}

    a2 = {i: (eff_l[i] ** 2).sum(0) for i in LHS_VIEWS}     # [8192] f64
    b2 = {j: (eff_r[j] ** 2).sum(0) for j in RHS_VIEWS}

    # aux fp8 rows; device-exact norm sums include the tiny fp8 residual
    g1, g2, g3 = {}, {}, {}
    a2_dev = {}
    for i in LHS_VIEWS:
        g1[i], g2[i], g3[i], res = _decomp3(a2[i], first_half=True)
        a2_dev[i] = a2[i] - res
    h1, h2, h3 = {}, {}, {}
    b2_dev = {}
    for j in RHS_VIEWS:
        h1[j], h2[j], h3[j], res = _decomp3(b2[j])
        b2_dev[j] = b2[j] - res

    rhs_tiles = {}
    for j in RHS_VIEWS:
        k1 = np.zeros((128, B), dtype=ml_dtypes.float8_e4m3)
        k1[0, :] = h1[j]; k1[1, :] = h2[j]; k1[2, :] = h3[j]
        k1[3, :] = 2.0;   k1[4, :] = 1.0;   k1[5, :] = 1.0
        rhs_tiles[j] = np.ascontiguousarray(np.stack([rhs0[j], k1], axis=1))

    lhs_tiles = {}
    for i in LHS_VIEWS:
        k1 = np.zeros((128, B), dtype=ml_dtypes.float8_e4m3)
        k1[0, :] = 1.0; k1[1, :] = 1.0; k1[2, :] = 1.0
        k1[3, :] = g1[i]; k1[4, :] = g2[i]; k1[5, :] = g3[i]
        lhs_tiles[i] = np.ascontiguousarray(np.stack([lhs0[i], k1], axis=1))

    # exact norms/dots of the ORIGINAL f32 inputs (f64 accumulation)
    nrm_true = [(t * t).sum(0) for t in zT64]

    # ---- positive-pair term: exact on host, O(B*D) ----
    pos_loss = sum(
        float(np.sqrt(np.maximum(
            nrm_true[i] + nrm_true[j] - 2.0 * (zT64[i] * zT64[j]).sum(0), 0.0)).mean())
        for i, j in PAIRS)

    # ---- calibration samples: device q-hat vs exact exp sums ----
    rng = np.random.default_rng(12345)
    NS = NCORES * 128 * CALN   # distinct samples per pair (128 rows per core)
    calqs = []                 # per core: [128, 3, CALN] f32
    true_sums = np.zeros((NCORES, 3))
    for p, (i, j) in enumerate(PAIRS):
        mi = rng.integers(0, B, size=NS)
        nj = rng.integers(0, B, size=NS)
        dot_eff = (eff_l[i][:, mi] * eff_r[j][:, nj]).sum(0)
        qhat = (a2_dev[i][mi] + b2_dev[j][nj] - 2.0 * dot_eff)
        dot_true = (zT64[i][:, mi] * zT64[j][:, nj]).sum(0)
        d_true = np.sqrt(np.maximum(
            nrm_true[i][mi] + nrm_true[j][nj] - 2.0 * dot_true, 0.0))
        ev = np.exp((S_SHIFT - d_true) / TAU)
        per = 128 * CALN
        for c in range(NCORES):
            sl = slice(c * per, (c + 1) * per)
            if p == 0:
                calqs.append(np.zeros((128, 3, CALN), dtype=np.float32))
            calqs[c][:, p, :] = np.float32(qhat[sl]).reshape(128, CALN)
            true_sums[c, p] = ev[sl].sum()

    in_maps = []
    for c in range(NCORES):
        m = {f"rhs{j}": rhs_tiles[j] for j in RHS_VIEWS}
        for i in LHS_VIEWS:
            m[f"lhs{i}"] = np.ascontiguousarray(lhs_tiles[i][:, :, c * ML:(c + 1) * ML])
        m["calq"] = calqs[c]
        in_maps.append(m)

    if "nc" not in _CACHE:
        nc = bacc.Bacc("TRN2", target_bir_lowering=False)
        build(nc)
        nc.finalize()
        _CACHE["nc"] = nc
    nc = _CACHE["nc"]

    res = None
    for attempt in range(4):
        try:
            res = run_bass_kernel_spmd(nc, in_maps, core_ids=list(range(NCORES)))
            outs = [r["out"] for r in res.results]
            cal_dev = np.array([[o[:, 24 + p].sum() for p in range(3)] for o in outs])
            ratios = cal_dev / true_sums
            ok = (np.all(np.isfinite(ratios)) and np.all(ratios > 0.7)
                  and np.all(ratios < 1.5)
                  and all(np.all(np.isfinite(o)) and np.all(o[:, :11] > 0)
                          and np.all(o[:, 12:22] > 0) and np.all(o[:, 27:35] > 0)
                          and np.all(o[:, 36:40] > 0) for o in outs))
        except Exception:
            ok = False
        if ok:
            break
        import time
        import jax
        try:
            jax.clear_backends()
        except Exception:
            pass
        time.sleep(8)
    assert res is not None
    _CACHE["last_res"] = res

    outs = [r["out"].astype(f64) for r in res.results]
    cal_dev = np.array([[o[:, 24 + p].sum() for p in range(3)] for o in outs])
    R = cal_dev.sum(0) / true_sums.sum(0)          # per-pair wiggle ratio

    # last two row-jobs drained chunk-wise: job 22 -> cols 31..34,
    # job 23 -> cols 27..30 plus col 35 (its final 512 slice)
    outs = [np.concatenate([o[:, :11], (o[:, 36:40].sum(1))[:, None], o[:, 12:22],
                            (o[:, 31:35].sum(1))[:, None],
                            (o[:, 27:31].sum(1) + o[:, 35])[:, None]], axis=1)
            for o in outs]
    neg_loss = 0.0
    for p in range(3):
        svals = np.concatenate([o[:, p * MB:(p + 1) * MB].reshape(-1) for o in outs])
        lse = np.log(svals) - math.log(R[p]) - S_SHIFT / TAU
        neg_loss += float(lse.mean()) - math.log(B)

    loss = (ALPHA * pos_loss + (1.0 - ALPHA) * neg_loss) / len(PAIRS)
    return np.float32(loss)
